# revision 1
# baseline (speedup 1.0000x reference)
"""Trainium2 kernel for nn_CompLinear3 (vq_codebook).

Strategy (column-parallel over out_features, per sharding hint):
- Host: layout prep (x transposed to [in,tok], decode of the VQ weight via the
  tiny MLP, de-standardization folded into the weight, bf16 cast, per-core
  column slicing of W/bias).
- Device (8 NeuronCores): the full [8192,4096]x[4096,4096] linear, each core
  computing its 512-column slice at the bf16 PE roofline: W slice resident in
  SBUF, x^T streamed once, PSUM accumulation over 32 K-tiles, bias added in
  the PSUM->SBUF epilogue on VectorE.
"""
import numpy as np
import ml_dtypes

IN_F = 4096
OUT_F = 4096
TOK = 8192
NCORES = 8
OPC = OUT_F // NCORES          # 512 out-features per core
KT = IN_F // 128               # 32 k-tiles
TT = TOK // 512                # 16 token chunks
OC = OPC // 128                # 4 psum column groups

_CACHE = {}


def _build():
    import concourse.bacc as bacc
    import concourse.mybir as mybir
    import concourse.tile as tile

    nc = bacc.Bacc("TRN2", target_bir_lowering=False, debug=False)
    xt = nc.dram_tensor("xt", [IN_F, TOK], mybir.dt.bfloat16, kind="ExternalInput")
    w = nc.dram_tensor("w", [IN_F, OPC], mybir.dt.bfloat16, kind="ExternalInput")
    bsc = nc.dram_tensor("bsc", [128, OC], mybir.dt.float32, kind="ExternalInput")
    out = nc.dram_tensor("o", [OPC, TOK], mybir.dt.float32, kind="ExternalOutput")

    wv = w[:].rearrange("(n p) o -> n p o", p=128)
    xv = xt[:].rearrange("(n p) t -> n p t", p=128)

    with tile.TileContext(nc) as tc:
        with tc.tile_pool(name="wp", bufs=1) as wp, \
             tc.tile_pool(name="xp", bufs=3) as xp, \
             tc.tile_pool(name="op", bufs=4) as op, \
             tc.tile_pool(name="ps", bufs=4, space="PSUM") as ps:
            bias_sb = wp.tile([128, OC], mybir.dt.float32)
            nc.sync.dma_start(bias_sb[:], bsc[:])
            w_sb = []
            for it in range(KT):
                t = wp.tile([128, OPC], mybir.dt.bfloat16, tag=f"w{it}")
                nc.sync.dma_start(t[:], wv[it])
                w_sb.append(t)
            for tchunk in range(TT):
                x_sb = []
                for it in range(KT):
                    t = xp.tile([128, 512], mybir.dt.bfloat16, tag=f"x{it}")
                    nc.sync.dma_start(
                        t[:], xv[it][:, tchunk * 512:(tchunk + 1) * 512])
                    x_sb.append(t)
                for oc in range(OC):
                    psum = ps.tile([128, 512], mybir.dt.float32, tag="ps")
                    for it in range(KT):
                        nc.tensor.matmul(
                            psum[:],
                            w_sb[it][:, oc * 128:(oc + 1) * 128],
                            x_sb[it][:],
                            start=(it == 0), stop=(it == KT - 1),
                        )
                    o_sb = op.tile([128, 512], mybir.dt.float32, tag="o")
                    nc.vector.tensor_scalar_add(
                        o_sb[:], psum[:], bias_sb[:, oc:oc + 1])
                    nc.sync.dma_start(
                        out[oc * 128:(oc + 1) * 128,
                            tchunk * 512:(tchunk + 1) * 512],
                        o_sb[:])
    nc.compile()
    return nc


def kernel(x, y_in_idx, codebook, W1, b1, W2, b2, scale, shift, bias):
    from concourse.bass_utils import run_bass_kernel_spmd

    x = np.asarray(x, np.float32)
    y_in_idx = np.asarray(y_in_idx).astype(np.int64)
    codebook = np.asarray(codebook, np.float32)
    W1 = np.asarray(W1, np.float32); b1 = np.asarray(b1, np.float32)
    W2 = np.asarray(W2, np.float32); b2 = np.asarray(b2, np.float32)
    scale = np.asarray(scale, np.float32); shift = np.asarray(shift, np.float32)
    bias = np.asarray(bias, np.float32)

    # Host layout prep + VQ decode (tiny MLP; the 275-GFLOP linear runs on device)
    codes = codebook[y_in_idx]                       # [NB, 16]
    h = np.maximum(codes @ W1 + b1, 0.0)             # [NB, 64]
    blocks = h @ W2 + b2                             # [NB, 16]
    W_hat = blocks.reshape(OUT_F, IN_F) * scale[:, None] + shift[:, None]

    xt = np.ascontiguousarray(x.reshape(TOK, IN_F).T).astype(ml_dtypes.bfloat16)

    if "nc" not in _CACHE:
        _CACHE["nc"] = _build()
    nc = _CACHE["nc"]

    in_maps = []
    for m in range(NCORES):
        wm = np.ascontiguousarray(
            W_hat[m * OPC:(m + 1) * OPC].T).astype(ml_dtypes.bfloat16)
        bm = np.ascontiguousarray(
            bias[m * OPC:(m + 1) * OPC].reshape(OC, 128).T).astype(np.float32)
        in_maps.append({"xt": xt, "w": wm, "bsc": bm})

    res = run_bass_kernel_spmd(nc, in_maps, core_ids=list(range(NCORES)))
    _CACHE["last_exec_ns"] = res.exec_time_ns

    full = np.concatenate([res.results[m]["o"] for m in range(NCORES)], axis=0)
    return np.ascontiguousarray(full.T).reshape(4, 2048, IN_F).astype(np.float32)
